# revision 34
# baseline (speedup 1.0000x reference)
"""Trainium2 Bass kernel for the Diversity4 loss.

Math (per sample b, models m=0..3, classes c=0..999):
    p_m = softmax(x_m / T);  v_m = (p_m - mean(p_m)) / ||p_m - mean(p_m)||
    d_b = sum_{j<k} v_j . v_k ;  answer = mean(SCALE * d_b)

Simplifications used:
  * Softmax normalization cancels:  v_m = (e_m - mean(e_m)) / ||e_m - mean(e_m)||
    with e_m = exp(x_m / T).
  * Pairwise-dot sum: d_b = 0.5 * (||s||^2 - sum_m ||v_m||^2),  s = sum_m v_m.
  * Each v_m is centered, so s is centered and with w = sum_m rnorm_m * e_m,
    ||s||^2 = sum_c (w - wbar)^2 (wbar = mean of w; variance is
    shift-invariant so no per-model mean correction is needed).
  * sum_m ||v_m||^2 (the tau correction) is computed explicitly rather than
    assumed == 4, so rsqrt bias cancels to first order.  Only its GLOBAL sum
    matters:  sum_b d_b = sum_b C*var(w_b) - C * sum_{b,m} var_m * rnorm_m^2,
    so tau reduces to cheap elementwise sums -- no extra matmul.
  * rnorm = exp(-0.5 * ln(C * var)): ln and exp live in one ACT table set
    (natural_log_exp_and_others), so the kernel never switches ACT tables
    (a switch costs ~2.7us).  Copy/Square are set-resident fillers.
  * The w-sum over models runs on the TensorEngine in float32r (~12-bit
    mantissa, 1 cycle/col vs 4 for fp32).  Rounding noise is zero-mean and
    contributes <0.1% to the final mean; the tau correction uses the same
    rounded rnorm so normalization stays self-consistent.

Layout: 8 cores x 2048 samples.  Per core, 16 groups of 512 rows of the
host-interleaved xall (row 4*s+m = model m of sample s).  Each group is one
[128, 4, 1000] SBUF tile: partition p = 4*s' + m (s' = s mod 32), free dim =
(tile j, class c).

Scheduling: the per-group work is split into a front half (exp, bn stats,
rnorm, tau -- ACT/DVE) and a back half (lhsT scatter, W/Wb matmuls, Square
accumulation -- Pool/PE/ACT/DVE).  Emission is software-pipelined as
loads(g+1), front(g), back(g-1): each engine's program order then never
blocks on the cross-engine round trip of its own group, so DMA (the 91us
roofline for the 32.75MB/core input) stays saturated and the drain tail
after the last load is just one group's chain.  The last group additionally
runs per-tile (fine) so the tail chain hangs off a single [128,1000] tile.

Final fold is on the host: the kernel stores accs[128,2] per core
(col 0 = sum over groups of sum_c (w-wbar)^2 per sample-partition,
col 1 = tau partial sums per model-partition) and the host computes
answer = SCALE * 0.5 * (sum accs[:,0] - C * sum accs[:,1]) / B.
"""

import sys

import numpy as np

for _p in ("/opt/trn_rl_repo",):
    if _p not in sys.path:
        sys.path.insert(0, _p)

import concourse.bacc as bacc
import concourse.tile as tile
from concourse import mybir
from concourse import bass_utils

B, C = 16384, 1000
N_CORES = 8
B_LOC = B // N_CORES  # 2048
SPT = 32  # samples per tile (x 4 models = 128 partitions)
GROUP = 4  # tiles per PSUM stack / load group
N_GROUPS = B_LOC // (SPT * GROUP)  # 16
T = 20.0
SCALE = 0.3
PE_WARM_N = 0
PE_WARM_FINE = 0

F32 = mybir.dt.float32
F32R = mybir.dt.float32r
AF = mybir.ActivationFunctionType
AX = mybir.AxisListType
AL = mybir.AluOpType


def _build_nc():
    nc = bacc.Bacc("TRN2")
    # xall rows are interleaved (sample-major): row 4*s + m = model m of sample s
    xall = nc.dram_tensor("xall", [B_LOC * 4, C], F32, kind="ExternalInput")
    out = nc.dram_tensor("partial", [128, 6], F32, kind="ExternalOutput")

    # Z0[p, s] = 1 iff s == p // 4: the 32-sample scatter mask.  Tile j's
    # lhsT band (cols 32j..32j+32) is Z0 * rnorm_j; the rest of each
    # persistent lhsT tile stays zero from a one-time memset.
    z_np = np.zeros((128, SPT), dtype=np.float32)
    z_np[np.arange(128), np.arange(128) // 4] = 1.0
    z_dram = nc.inline_tensor(np.ascontiguousarray(z_np), name="z0")

    with tile.TileContext(nc) as tc:
        with (
            tc.tile_pool(name="singles", bufs=1) as singles,
            tc.tile_pool(name="xpool", bufs=5) as xpool,
            tc.tile_pool(name="epool", bufs=4) as epool,
            tc.tile_pool(name="small", bufs=6) as small,
            tc.tile_pool(name="wpsum", bufs=3, space="PSUM") as wpsum,
            tc.tile_pool(name="psing", bufs=1, space="PSUM") as psing,
        ):
            def emit_loads(g):
                r0 = g * GROUP * 128
                X = xpool.tile([128, GROUP, C], F32, tag="X")
                for j in range(GROUP):
                    nc.sync.dma_start(
                        out=X[:, j, :],
                        in_=xall[r0 + 128 * j : r0 + 128 * (j + 1), :],
                    )
                return X

            # Group-0 loads go first so the input stream starts immediately;
            # everything below is off the DMA critical path.
            X_live = {0: emit_loads(0)}

            z0 = singles.tile([128, SPT], F32)
            nc.gpsimd.dma_start(out=z0, in_=z_dram[:, :])
            ones = singles.tile([128, 1], F32)
            nc.vector.memset(ones, 1.0)
            warm = singles.tile([128, 1], F32)
            nc.scalar.activation(warm, ones, AF.Exp)
            # accs[:,0] = sum_c (w-wbar)^2 partials; accs[:,1:5] = tau
            # partials per tile slot (host sums the columns -- no on-device
            # reduce needed); accs[:,5] = var(w) partials from the bn path
            # (last group only).
            accs = singles.tile([128, 6], F32)
            nc.vector.memset(accs, 0.0)
            # Per-tile scatter masks zm[j][p, 32j + p//4] = 1: built from z0
            # by four tiny one-time band DMAs.  The full-width lhsT writes
            # they feed stagger matmul readiness enough that the PE p-state
            # model sees a warm engine (a band-only write makes all matmuls
            # ready at once and they all charge the cold-PE rate).
            zms = []
            lts = []
            for j in range(GROUP):
                zm = singles.tile([128, 128], F32, tag=f"zm{j}")
                nc.vector.memset(zm, 0.0)
                nc.gpsimd.dma_start(
                    out=zm[:, SPT * j : SPT * (j + 1)], in_=z_dram[:, :]
                )
                zms.append(zm)
                lt = singles.tile([128, 128], F32R, tag=f"lt{j}")
                lts.append(lt)
            lt2 = singles.tile([128, 128], F32, tag="ltwb")
            nc.vector.memset(lt2, 0.0)
            # wbar lives in a persistent PSUM tile; column g%3 covers the
            # two-group gap between the Wb matmul and the Square that reads
            # it as bias.
            wbp = psing.tile([128, 8], F32)
            # PE keep-warm: the p-state model charges 3.7x for matmuls issued
            # to a cold engine (idle gaps between group bursts reset the
            # ramp).  Filler matmuls into a junk PSUM bank keep the engine
            # continuously busy so every real matmul runs at the full rate.
            djunk = singles.tile([128, 512], F32R, tag="djunk")
            nc.vector.memset(djunk.bitcast(F32), 0.0)
            dpsum = psing.tile([1, 512], F32, tag="dpsum")
            ones_r = ones.bitcast(F32R)

            def pe_warm(n):
                for _ in range(n):
                    nc.tensor.matmul(
                        dpsum, ones_r, djunk, start=True, stop=True,
                        skip_group_check=True,
                    )

            def mid(g, X):
                """In-group work: exp, stats, rnorm, tau, lhsT bands, W/Wb
                matmuls.  Everything here is latency-critical for group g;
                the W-gated Square is deferred two groups (see back)."""
                fine = g >= N_GROUPS - 2
                E = epool.tile([128, GROUP, C], F32R, tag="E")
                Ef = E.bitcast(F32)
                last_grp = g == N_GROUPS - 1
                if fine:
                    for j in range(GROUP):
                        if last_grp and j == GROUP - 1:
                            # split the very last tile so its stats chain
                            # starts half a tile earlier
                            for h in range(2):
                                nc.scalar.activation(
                                    E[:, j, 500 * h : 500 * (h + 1)],
                                    X[:, j, 500 * h : 500 * (h + 1)],
                                    AF.Exp, scale=1.0 / T,
                                )
                        else:
                            nc.scalar.activation(
                                E[:, j, :], X[:, j, :], AF.Exp, scale=1.0 / T
                            )
                else:
                    nc.scalar.activation(E, X, AF.Exp, scale=1.0 / T)
                stats = small.tile([128, 2 * GROUP, 6], F32, tag="stats")
                mv = small.tile([128, GROUP, 2], F32, tag="mv")
                L = small.tile([128, GROUP], F32, tag="L")
                rn = small.tile([128, GROUP], F32R, tag="rn")
                rnf = rn.bitcast(F32)
                W = wpsum.tile([128, 1024], F32, tag="W")
                need_wb = not last_grp  # last group takes the bn-var path
                rm = None
                if need_wb:
                    rm = small.tile([128, GROUP], F32, tag="rm")
                # --- stats + rnorm + scatter + matmuls ---
                if fine:
                    for j in range(GROUP):
                        for h in range(2):
                            nc.vector.bn_stats(
                                stats[:, 2 * j + h, :],
                                Ef[:, j, 500 * h : 500 * (h + 1)],
                            )
                        nc.vector.bn_aggr(
                            mv[:, j, :], stats[:, 2 * j : 2 * j + 2, :]
                        )
                        nc.scalar.activation(
                            L[:, j : j + 1], mv[:, j, 1:2], AF.Ln,
                            scale=float(C),
                        )
                        nc.scalar.activation(
                            rn[:, j : j + 1], L[:, j : j + 1], AF.Exp,
                            scale=-0.5,
                        )
                        if need_wb:
                            nc.gpsimd.tensor_mul(
                                rm[:, j : j + 1], rnf[:, j : j + 1],
                                mv[:, j, 0:1],
                            )
                        band = slice(SPT * j, SPT * (j + 1))
                        with nc.allow_low_precision(reason="f32r lhsT for PE"):
                            nc.gpsimd.tensor_scalar_mul(
                                lts[j], zms[j], rnf[:, j : j + 1]
                            )
                        if need_wb:
                            nc.gpsimd.tensor_scalar_mul(
                                lt2[:, band], z0, rm[:, j : j + 1]
                            )
                        first, last = (j == 0), (j == GROUP - 1)
                        if need_wb and last:
                            c = g % 3
                            nc.tensor.matmul(
                                wbp[:, c : c + 1], lt2, ones,
                                start=True, stop=True,
                            )
                        nc.tensor.matmul(
                            W[:, 0:512], lts[j], E[:, j, 0:512],
                            start=first, stop=last, skip_group_check=True,
                        )
                        nc.tensor.matmul(
                            W[:, 512:1000], lts[j], E[:, j, 512:1000],
                            start=first, stop=last, skip_group_check=True,
                        )
                        if not last:
                            pe_warm(PE_WARM_FINE)
                        if last_grp and last:
                            # var(w) on DVE, interleaved with the W halves
                            wstats = small.tile([128, 2, 6], F32, tag="wstats")
                            wmv = small.tile([128, 2], F32, tag="wmv")
                            nc.vector.bn_stats(wstats[:, 0, :], W[:, 0:512])
                            nc.vector.bn_stats(wstats[:, 1, :], W[:, 512:1000])
                            nc.vector.bn_aggr(wmv, wstats)
                            nc.vector.tensor_add(
                                accs[:, 5:6], accs[:, 5:6], wmv[:, 1:2]
                            )
                if not fine:
                    Ev = Ef.rearrange("p j (h x) -> p (j h) x", h=2)
                    for h in range(2 * GROUP):
                        nc.vector.bn_stats(stats[:, h, :], Ev[:, h, :])
                    for j in range(GROUP):
                        nc.vector.bn_aggr(
                            mv[:, j, :], stats[:, 2 * j : 2 * j + 2, :]
                        )
                    nc.scalar.activation(L, mv[:, :, 1], AF.Ln, scale=float(C))
                    nc.scalar.activation(rn, L, AF.Exp, scale=-0.5)
                    nc.gpsimd.tensor_mul(rm, rnf, mv[:, :, 0])
                    for j in range(GROUP):
                        band = slice(SPT * j, SPT * (j + 1))
                        with nc.allow_low_precision(reason="f32r lhsT for PE"):
                            nc.gpsimd.tensor_scalar_mul(
                                lts[j], zms[j], rnf[:, j : j + 1]
                            )
                        nc.gpsimd.tensor_scalar_mul(
                            lt2[:, band], z0, rm[:, j : j + 1]
                        )
                        first, last = (j == 0), (j == GROUP - 1)
                        nc.tensor.matmul(
                            W[:, 0:512], lts[j], E[:, j, 0:512],
                            start=first, stop=last, skip_group_check=True,
                        )
                        nc.tensor.matmul(
                            W[:, 512:1000], lts[j], E[:, j, 512:1000],
                            start=first, stop=last, skip_group_check=True,
                        )
                    c = g % 3
                    nc.tensor.matmul(
                        wbp[:, c : c + 1], lt2, ones, start=True, stop=True
                    )
                    pe_warm(PE_WARM_N)
                wb = None
                if need_wb:
                    # wbar to SBUF on DVE (ACT bias operands must be SBUF;
                    # doing the copy here keeps it off the ACT stream)
                    c = g % 3
                    wb = small.tile([128, 1], F32, tag="wb")
                    nc.vector.tensor_scalar(
                        wb, wbp[:, c : c + 1], 1.0, None, op0=AL.mult
                    )
                # tau partials: t = var * rnorm^2; only sum_{p,j} t is needed.
                # All on Pool: it has slack, DVE and ACT do not.  Accumulated
                # column-wise; the host sums the 4 columns at the end.
                tg = small.tile([128, GROUP], F32, tag="tg")
                nc.gpsimd.tensor_mul(tg, rnf, rnf)
                nc.gpsimd.tensor_mul(tg, tg, mv[:, :, 1])
                nc.gpsimd.tensor_add(accs[:, 1:5], accs[:, 1:5], tg)
                return W, wb

            def back(g, W, wb):
                """Deferred two groups: Square((wbar - w)^2) + accumulate.
                By emission time W and wbar are long ready, so these never
                clog the ACT wait queue.  One full-width Square (PSUM banks
                are linearly addressable for ACT reads; only matmul
                accumulation is bank-scoped)."""
                sqscr = small.tile([128, 1024], F32, tag="sqscr", bufs=2)
                q = small.tile([128, 2], F32, tag="q")
                nc.scalar.activation(
                    sqscr[:, 0:1000], W[:, 0:1000], AF.Square,
                    scale=-1.0, bias=wb, accum_out=q[:, 0:1],
                )
                nc.gpsimd.tensor_add(accs[:, 0:1], accs[:, 0:1], q[:, 0:1])

            W_live = {}
            for g in range(N_GROUPS):
                if g + 1 < N_GROUPS:
                    X_live[g + 1] = emit_loads(g + 1)
                W_live[g] = mid(g, X_live.pop(g))
                if g - 2 >= 0:
                    back(g - 2, *W_live.pop(g - 2))
            back(N_GROUPS - 2, *W_live.pop(N_GROUPS - 2))
            nc.sync.dma_start(out=out[:, :], in_=accs)
    _strip_redundant_dma_waits(nc)
    # Force the ACT table chooser onto the one set that serves every function
    # this kernel uses (exp, ln, square, copy, identity).  The default greedy
    # chooser picks exp_and_others for Exp and natural_log for Ln, inserting
    # a ~2.7us table swap per activation pair, ~86us/core of pure overhead.
    _orig_tables = bacc.get_activation_tables

    def _only_shared(arch):
        tabs = _orig_tables(arch)
        return {
            name: (fns if name == "natural_log_exp_and_others" else set())
            for name, fns in tabs.items()
        }

    bacc.get_activation_tables = _only_shared
    try:
        nc.finalize()
    finally:
        bacc.get_activation_tables = _orig_tables
    return nc


def _strip_redundant_dma_waits(nc):
    """Drop same-queue WAW waits on load DMAs.

    The HWDGE DMA pseudo-instruction accepts a single sync wait, but Tile
    emits two once SBUF slots recycle: [engine-sem release by the slot's
    reader, own-queue WAW vs the slot's previous writer].  The WAW wait is
    redundant when the kept reader wait transitively orders the new write
    after the old one.  Only remove a wait that targets the exact semaphore
    this DMA updates (same queue) while an engine-semaphore wait remains;
    anything else is left for Bacc's event-semaphore splitting.
    """
    removed = 0
    for inst in nc.inst_map.values():
        if type(inst).__name__ != "InstDMACopy":
            continue
        si = getattr(inst, "sync_info", None)
        if si is None:
            continue
        waits = list(si.on_wait or [])
        if len(waits) <= 1:
            continue
        upd_names = {u.ant_name for u in (si.on_update or [])}
        drop = [w for w in waits if w.ant_name in upd_names]
        keep = [w for w in waits if w.ant_name not in upd_names]
        if len(drop) != 1 or not keep:
            continue
        if any(k.ant_name.startswith("DMA") for k in keep):
            continue
        si.on_wait = keep
        inst.sync_info = si
        removed += 1
    return removed


_NC_CACHE = {}


def _get_nc():
    if "nc" not in _NC_CACHE:
        _NC_CACHE["nc"] = _build_nc()
    return _NC_CACHE["nc"]


def run_on_cores(arrays, trace=False):
    """arrays: list of 4 full [B, C] f32 arrays. Returns (partials, results)."""
    nc = _get_nc()
    # Interleave models per sample: xall[4*s + m, c] = arrays[m][s, c]
    xall = np.ascontiguousarray(
        np.stack(arrays, axis=1).reshape(B * 4, C).astype(np.float32, copy=False)
    )
    in_maps = []
    for k in range(N_CORES):
        in_maps.append({"xall": xall[k * B_LOC * 4 : (k + 1) * B_LOC * 4]})
    res = bass_utils.run_bass_kernel_spmd(
        nc, in_maps, core_ids=list(range(N_CORES)), trace=trace
    )
    # per-core partial = accs [128, 6] (fold on host)
    partials = [np.asarray(r["partial"], dtype=np.float64) for r in res.results]
    return partials, res


def kernel(outputs1, outputs2, outputs3, outputs4, targets=None):
    arrays = [
        np.ascontiguousarray(np.asarray(a, dtype=np.float32))
        for a in (outputs1, outputs2, outputs3, outputs4)
    ]
    partials, _ = run_on_cores(arrays, trace=False)
    tot = np.zeros(6, dtype=np.float64)
    for p in partials:
        tot += p.sum(axis=0)
    # col 0: sum (w-wbar)^2; cols 1-4: tau; col 5: bn-path var(w) (unscaled)
    d_sum = tot[0] + C * tot[5] - C * (tot[1] + tot[2] + tot[3] + tot[4])
    ans = SCALE * 0.5 * d_sum / B
    return np.array(ans, dtype=np.float32)
